# revision 8
# baseline (speedup 1.0000x reference)
"""Trainium2 Bass kernel for MoEPred: softmax-gated mixture of 32 tiny experts.

  xi[b] = sum_e softmax_e(x@Wg.T) * (W2[e] . gelu(x @ W1[e] + b1[e]) + b2[e])

Sharding: pure data parallel over batch across 8 NeuronCores. x is
pre-transposed on the host so the feature (contraction) dim lands on SBUF
partitions; all compute happens in "transposed space" ([feat/eh, rows]):

  per 512-row macro-tile, per core:
    hT   [512eh, R] = W1flat.T @ xT          (16 f32r matmuls, K=128, N=512)
    haT             = gelu(hT + b1)          (ACT, fused bias, PSUM->SBUF)
    gT   [32, R]    = Wg @ xT                (4 matmuls)
    o2T  [32, R]    = W2blockdiag.T @ haT    (4 matmuls)
    st[0:32]  = exp(gT)                      (ACT)
    st[32:64] = (o2T + b2) * st[0:32]        (DVE scalar_tensor_tensor)
    sums [2, R] = sel.T @ st                 (ones-matmul partition reduction)
    xiT  [1, R] = sums[1] / sums[0]          (DVE reciprocal + mul)
"""

import os
import sys
from contextlib import ExitStack

import numpy as np

for _p in ("/opt/trn_rl_repo",):
    if _p not in sys.path:
        sys.path.insert(0, _p)

import jax
from jax.experimental.shard_map import shard_map
from jax.sharding import Mesh, NamedSharding, PartitionSpec

import concourse.bacc as bacc
import concourse.bass2jax as b2j
import concourse.tile as tile
from concourse import mybir

N_CORES = 8
BATCH = 262144
D_IN = 512
N_EXPERTS = 32
HID = 16
EH = N_EXPERTS * HID  # 512
B_LOC = BATCH // N_CORES  # 32768
R = 512  # rows per macro-tile
KC = D_IN // 128  # 4 feature chunks
MC = EH // 128  # 4 eh chunks

F32 = mybir.dt.float32
F32R = mybir.dt.float32r
AF = mybir.ActivationFunctionType
ALU = mybir.AluOpType

LAST_RESULT = None  # BassKernelResults of the most recent run (for test harness)
_NC_CACHE = {}


def build_nc(b_loc=B_LOC, act_fn=None):
    if act_fn is None:
        act_fn = AF.Gelu
    assert b_loc % R == 0
    n_macro = b_loc // R

    nc = bacc.Bacc("TRN2", target_bir_lowering=False, debug=False, num_devices=N_CORES)

    xT = nc.dram_tensor("xT", [D_IN, b_loc], F32R, kind="ExternalInput")
    w1t = nc.dram_tensor("w1t", [D_IN, EH], F32R, kind="ExternalInput")
    wgt = nc.dram_tensor("wgt", [D_IN, N_EXPERTS], F32R, kind="ExternalInput")
    w2bd = nc.dram_tensor("w2bd", [EH, N_EXPERTS], F32R, kind="ExternalInput")
    b1c = nc.dram_tensor("b1c", [128, MC], F32, kind="ExternalInput")
    b2c = nc.dram_tensor("b2c", [N_EXPERTS, 1], F32, kind="ExternalInput")
    sel = nc.dram_tensor("sel", [64, 33], F32R, kind="ExternalInput")
    outT = nc.dram_tensor("outT", [1, b_loc], F32, kind="ExternalOutput")

    with tile.TileContext(nc) as tc, ExitStack() as ctx:
        const = ctx.enter_context(tc.tile_pool(name="const", bufs=1))
        xpool = ctx.enter_context(tc.tile_pool(name="xp", bufs=3))
        hapool = ctx.enter_context(tc.tile_pool(name="hap", bufs=2))
        stpool = ctx.enter_context(tc.tile_pool(name="stp", bufs=2))
        xopool = ctx.enter_context(tc.tile_pool(name="xop", bufs=4))
        ps_h = ctx.enter_context(tc.tile_pool(name="ps_h", bufs=2, space="PSUM"))
        ps_g = ctx.enter_context(tc.tile_pool(name="ps_g", bufs=2, space="PSUM"))
        ps_o = ctx.enter_context(tc.tile_pool(name="ps_o", bufs=2, space="PSUM"))
        ps_s = ctx.enter_context(tc.tile_pool(name="ps_s", bufs=2, space="PSUM"))

        # --- replicated constants, loaded once ---
        w1_sb = const.tile([128, KC * EH], F32R, name="w1_sb")
        wg_sb = const.tile([128, KC * N_EXPERTS], F32R, name="wg_sb")
        w2_sb = const.tile([128, MC * N_EXPERTS], F32R, name="w2_sb")
        b1_sb = const.tile([128, MC], F32, name="b1_sb")
        b2_sb = const.tile([N_EXPERTS, 1], F32, name="b2_sb")
        sel_sb = const.tile([64, 33], F32R, name="sel_sb")
        for k in range(KC):
            nc.sync.dma_start(w1_sb[:, k * EH:(k + 1) * EH], w1t[k * 128:(k + 1) * 128, :])
            nc.sync.dma_start(wg_sb[:, k * 32:(k + 1) * 32], wgt[k * 128:(k + 1) * 128, :])
            nc.sync.dma_start(w2_sb[:, k * 32:(k + 1) * 32], w2bd[k * 128:(k + 1) * 128, :])
        nc.sync.dma_start(b1_sb[:], b1c[:, :])
        nc.sync.dma_start(b2_sb[:], b2c[:, :])
        nc.sync.dma_start(sel_sb[:], sel[:, :])

        for j in range(n_macro):
            c0 = j * R
            xq = xpool.tile([128, KC * R], F32R, tag="xq", name="xq")
            for k in range(KC):
                nc.sync.dma_start(
                    xq[:, k * R:(k + 1) * R],
                    xT[k * 128:(k + 1) * 128, c0:c0 + R],
                )

            # hT = W1flat.T @ xT, then gelu(. + b1) -> ha
            ha = hapool.tile([128, MC * R], F32R, tag="ha", name="ha")
            for m in range(MC):
                hp = ps_h.tile([128, R], F32, tag="hp", name="hp")
                for k in range(KC):
                    nc.tensor.matmul(
                        hp[:],
                        lhsT=w1_sb[:, k * EH + m * 128: k * EH + (m + 1) * 128],
                        rhs=xq[:, k * R:(k + 1) * R],
                        start=(k == 0),
                        stop=(k == KC - 1),
                    )
                nc.scalar.activation(
                    ha[:, m * R:(m + 1) * R], hp[:], act_fn,
                    bias=b1_sb[:, m:m + 1], scale=1.0,
                )

            # gating logits gT = Wg @ xT
            gp = ps_g.tile([N_EXPERTS, R], F32, tag="gp", name="gp")
            for k in range(KC):
                nc.tensor.matmul(
                    gp[:],
                    lhsT=wg_sb[:, k * 32:(k + 1) * 32],
                    rhs=xq[:, k * R:(k + 1) * R],
                    start=(k == 0),
                    stop=(k == KC - 1),
                )

            # expert outputs o2T = W2blockdiag.T @ haT
            op = ps_o.tile([N_EXPERTS, R], F32, tag="op", name="op")
            for m in range(MC):
                nc.tensor.matmul(
                    op[:],
                    lhsT=w2_sb[:, m * 32:(m + 1) * 32],
                    rhs=ha[:, m * R:(m + 1) * R],
                    start=(m == 0),
                    stop=(m == MC - 1),
                )

            # st[0:32] = exp(g); st[32:64] = (o2 + b2) * exp(g)
            st = stpool.tile([64, R], F32R, tag="st", name="st")
            nc.scalar.activation(st[0:32, :], gp[:], AF.Exp)
            nc.vector.scalar_tensor_tensor(
                st[32:64, :], op[:], b2_sb[:], st[0:32, :], ALU.add, ALU.mult,
            )

            # partition-reduce: sums[0]=sum(exp), sums[1]=sum((o2+b2)*exp)
            sp = ps_s.tile([33, R], F32, tag="sp", name="sp")
            nc.tensor.matmul(
                sp[:], lhsT=sel_sb[:], rhs=st[:],
                start=True, stop=True,
            )

            rc = xopool.tile([1, R], F32, tag="rc", name="rc")
            xo = xopool.tile([1, R], F32, tag="xo", name="xo")
            nc.vector.reciprocal(rc[:], sp[0:1, :])
            nc.vector.tensor_mul(xo[:], sp[32:33, :], rc[:])
            nc.sync.dma_start(outT[0:1, c0:c0 + R], xo[:])

    nc.compile()
    return nc


def prep_weights(Wg, W1, b1, W2, b2):
    w1t = np.ascontiguousarray(
        np.asarray(W1, dtype=np.float32).transpose(1, 0, 2).reshape(D_IN, EH))
    wgt = np.ascontiguousarray(np.asarray(Wg, dtype=np.float32).T)
    w2bd = np.zeros((EH, N_EXPERTS), np.float32)
    W2 = np.asarray(W2, dtype=np.float32)
    for e in range(N_EXPERTS):
        w2bd[e * HID:(e + 1) * HID, e] = W2[e]
    b1c = np.ascontiguousarray(
        np.asarray(b1, dtype=np.float32).reshape(EH).reshape(MC, 128).T)
    b2c = np.asarray(b2, dtype=np.float32).reshape(N_EXPERTS, 1)
    selm = np.zeros((64, 33), np.float32)
    selm[0:32, 0] = 1.0
    selm[32:64, 32] = 1.0
    return {"w1t": w1t, "wgt": wgt, "w2bd": w2bd, "b1c": b1c, "b2c": b2c,
            "sel": selm}


class Runner:
    """Reusable SPMD executor: the multi-core path of
    concourse.bass2jax.run_bass_via_pjrt, factored so the jitted callable and
    device-resident inputs can be reused across calls (for benchmarking)."""

    def __init__(self, nc, n_cores=N_CORES):
        b2j.install_neuronx_cc_hook()
        self.nc = nc
        self.n_cores = n_cores
        partition_name = (
            nc.partition_id_tensor.name if nc.partition_id_tensor else None
        )
        in_names, out_names, out_avals, zero_outs = [], [], [], []
        for alloc in nc.m.functions[0].allocations:
            if not isinstance(alloc, mybir.MemoryLocationSet):
                continue
            assert alloc.memorylocations
            name = alloc.memorylocations[0].name
            if alloc.kind == "ExternalInput":
                if name != partition_name:
                    in_names.append(name)
            elif alloc.kind == "ExternalOutput":
                out_names.append(name)
                shape = tuple(alloc.tensor_shape)
                dtype = mybir.dt.np(alloc.dtype)
                out_avals.append(jax.core.ShapedArray(shape, dtype))
                zero_outs.append(np.zeros(shape, dtype))
        self.in_names = list(in_names)
        self.out_names = out_names
        self.zero_outs = zero_outs
        n_params = len(in_names)
        n_outs = len(out_names)
        bind_names = in_names + out_names
        if partition_name is not None:
            bind_names.append(partition_name)

        def _body(*args):
            operands = list(args)
            if partition_name is not None:
                operands.append(b2j.partition_id_tensor())
            outs = b2j._bass_exec_p.bind(
                *operands,
                out_avals=tuple(out_avals),
                in_names=tuple(bind_names),
                out_names=tuple(out_names),
                lowering_input_output_aliases=(),
                sim_require_finite=True,
                sim_require_nnan=True,
                nc=nc,
            )
            return tuple(outs)

        devices = jax.devices()[:n_cores]
        assert len(devices) == n_cores
        self.mesh = Mesh(np.asarray(devices), ("core",))
        in_specs = (PartitionSpec("core"),) * (n_params + n_outs)
        out_specs = (PartitionSpec("core"),) * n_outs
        self.fn = jax.jit(
            shard_map(_body, mesh=self.mesh, in_specs=in_specs,
                      out_specs=out_specs, check_rep=False),
            donate_argnums=tuple(range(n_params, n_params + n_outs)),
            keep_unused=True,
        )
        self.sharding = NamedSharding(self.mesh, PartitionSpec("core"))

    def put_inputs(self, in_maps):
        assert len(in_maps) == self.n_cores
        concat = [
            np.concatenate([np.asarray(m[name]) for m in in_maps], axis=0)
            for name in self.in_names
        ]
        return [jax.device_put(a, self.sharding) for a in concat]

    def fresh_outs(self):
        return [
            jax.device_put(
                np.zeros((self.n_cores * z.shape[0], *z.shape[1:]), z.dtype),
                self.sharding,
            )
            for z in self.zero_outs
        ]

    def run(self, dev_inputs, dev_outs=None):
        if dev_outs is None:
            dev_outs = self.fresh_outs()
        return self.fn(*dev_inputs, *dev_outs)


_RUNNER_CACHE = {}


def get_runner(b_loc=B_LOC):
    if b_loc not in _RUNNER_CACHE:
        if b_loc not in _NC_CACHE:
            _NC_CACHE[b_loc] = build_nc(b_loc)
        _RUNNER_CACHE[b_loc] = Runner(_NC_CACHE[b_loc])
    return _RUNNER_CACHE[b_loc]


def make_in_maps(x, Wg, W1, b1, W2, b2):
    x = np.asarray(x, dtype=np.float32)
    consts = prep_weights(Wg, W1, b1, W2, b2)
    xs = x.reshape(N_CORES, B_LOC, D_IN)
    in_maps = []
    for i in range(N_CORES):
        m = dict(consts)
        m["xT"] = np.ascontiguousarray(xs[i].T)
        in_maps.append(m)
    return in_maps


def kernel(x, Wg, W1, b1, W2, b2):
    os.environ["BASS_NEVER_TRACE"] = "1"
    in_maps = make_in_maps(x, Wg, W1, b1, W2, b2)
    runner = get_runner(B_LOC)
    dev_in = runner.put_inputs(in_maps)
    outs = runner.run(dev_in)
    out_t = np.asarray(outs[0])  # [N_CORES * 1, B_LOC]
    return np.ascontiguousarray(out_t.reshape(BATCH, 1))


if __name__ == "__main__":
    rng = np.random.default_rng(0)
    inputs = {
        "x": rng.standard_normal((BATCH, D_IN), dtype=np.float32),
        "Wg": (rng.standard_normal((N_EXPERTS, D_IN)) * 0.02).astype(np.float32),
        "W1": (rng.standard_normal((N_EXPERTS, D_IN, HID)) * 0.02).astype(np.float32),
        "b1": (rng.standard_normal((N_EXPERTS, HID)) * 0.02).astype(np.float32),
        "W2": (rng.standard_normal((N_EXPERTS, HID)) * 0.02).astype(np.float32),
        "b2": (rng.standard_normal((N_EXPERTS,)) * 0.02).astype(np.float32),
    }
    out = kernel(**inputs)
    print(out.shape, out.dtype, out[:4, 0])


# revision 10
# speedup vs baseline: 150.4061x; 150.4061x over previous
"""Trainium2 Bass kernel for MoEPred: softmax-gated mixture of 32 tiny experts.

  xi[b] = sum_e softmax_e(x@Wg.T) * (W2[e] . gelu(x @ W1[e] + b1[e]) + b2[e])

Sharding: pure data parallel over batch across 8 NeuronCores. x is
pre-transposed on the host so the feature (contraction) dim lands on SBUF
partitions; all compute happens in "transposed space" ([feat/eh, rows]):

  per 512-row macro-tile, per core:
    hT   [512eh, R] = W1flat.T @ xT          (16 f32r matmuls, K=128, N=512)
    haT             = gelu(hT + b1)          (ACT, fused bias, PSUM->SBUF)
    gT   [32, R]    = Wg @ xT                (4 matmuls)
    o2T  [32, R]    = W2blockdiag.T @ haT    (4 matmuls)
    st[0:32]  = exp(gT)                      (ACT)
    st[32:64] = (o2T + b2) * st[0:32]        (DVE scalar_tensor_tensor)
    sums [2, R] = sel.T @ st                 (ones-matmul partition reduction)
    xiT  [1, R] = sums[1] / sums[0]          (DVE reciprocal + mul)
"""

import os
import sys
from contextlib import ExitStack

import numpy as np

for _p in ("/opt/trn_rl_repo",):
    if _p not in sys.path:
        sys.path.insert(0, _p)

import jax
from jax.experimental.shard_map import shard_map
from jax.sharding import Mesh, NamedSharding, PartitionSpec

import concourse.bacc as bacc
import concourse.bass2jax as b2j
import concourse.tile as tile
from concourse import mybir

N_CORES = 8
BATCH = 262144
D_IN = 512
N_EXPERTS = 32
HID = 16
EH = N_EXPERTS * HID  # 512
B_LOC = BATCH // N_CORES  # 32768
R = 512  # rows per macro-tile
KC = D_IN // 128  # 4 feature chunks
MC = EH // 128  # 4 eh chunks

F32 = mybir.dt.float32
F32R = mybir.dt.float32r
AF = mybir.ActivationFunctionType
ALU = mybir.AluOpType

LAST_RESULT = None  # BassKernelResults of the most recent run (for test harness)
_NC_CACHE = {}


def build_nc(b_loc=B_LOC, act_fn=None, loop_n=1):
    """loop_n > 1 wraps the whole macro loop in a hardware For_i that redoes
    the identical work loop_n times — used only to amplify exec time above the
    ~99ms axon dispatch floor for benchmarking."""
    if act_fn is None:
        act_fn = AF.Gelu
    assert b_loc % R == 0
    n_macro = b_loc // R

    nc = bacc.Bacc("TRN2", target_bir_lowering=False, debug=False, num_devices=N_CORES)

    xT = nc.dram_tensor("xT", [D_IN, b_loc], F32R, kind="ExternalInput")
    w1t = nc.dram_tensor("w1t", [D_IN, EH], F32R, kind="ExternalInput")
    wgt = nc.dram_tensor("wgt", [D_IN, N_EXPERTS], F32R, kind="ExternalInput")
    w2bd = nc.dram_tensor("w2bd", [EH, N_EXPERTS], F32R, kind="ExternalInput")
    b1c = nc.dram_tensor("b1c", [128, MC], F32, kind="ExternalInput")
    b2c = nc.dram_tensor("b2c", [N_EXPERTS, 1], F32, kind="ExternalInput")
    sel = nc.dram_tensor("sel", [64, 33], F32R, kind="ExternalInput")
    outT = nc.dram_tensor("outT", [1, b_loc], F32, kind="ExternalOutput")

    with tile.TileContext(nc) as tc, ExitStack() as ctx:
        const = ctx.enter_context(tc.tile_pool(name="const", bufs=1))
        xpool = ctx.enter_context(tc.tile_pool(name="xp", bufs=3))
        hapool = ctx.enter_context(tc.tile_pool(name="hap", bufs=2))
        stpool = ctx.enter_context(tc.tile_pool(name="stp", bufs=2))
        xopool = ctx.enter_context(tc.tile_pool(name="xop", bufs=4))
        ps_h = ctx.enter_context(tc.tile_pool(name="ps_h", bufs=2, space="PSUM"))
        ps_g = ctx.enter_context(tc.tile_pool(name="ps_g", bufs=2, space="PSUM"))
        ps_o = ctx.enter_context(tc.tile_pool(name="ps_o", bufs=2, space="PSUM"))
        ps_s = ctx.enter_context(tc.tile_pool(name="ps_s", bufs=2, space="PSUM"))

        # --- replicated constants, loaded once ---
        w1_sb = const.tile([128, KC * EH], F32R, name="w1_sb")
        wg_sb = const.tile([128, KC * N_EXPERTS], F32R, name="wg_sb")
        w2_sb = const.tile([128, MC * N_EXPERTS], F32R, name="w2_sb")
        b1_sb = const.tile([128, MC], F32, name="b1_sb")
        b2_sb = const.tile([N_EXPERTS, 1], F32, name="b2_sb")
        sel_sb = const.tile([64, 33], F32R, name="sel_sb")
        for k in range(KC):
            nc.sync.dma_start(w1_sb[:, k * EH:(k + 1) * EH], w1t[k * 128:(k + 1) * 128, :])
            nc.sync.dma_start(wg_sb[:, k * 32:(k + 1) * 32], wgt[k * 128:(k + 1) * 128, :])
            nc.sync.dma_start(w2_sb[:, k * 32:(k + 1) * 32], w2bd[k * 128:(k + 1) * 128, :])
        nc.sync.dma_start(b1_sb[:], b1c[:, :])
        nc.sync.dma_start(b2_sb[:], b2c[:, :])
        nc.sync.dma_start(sel_sb[:], sel[:, :])

        loop_cm = tc.For_i(0, loop_n, 1) if loop_n > 1 else None
        if loop_cm is not None:
            ctx.enter_context(loop_cm)

        for j in range(n_macro):
            c0 = j * R
            xq = xpool.tile([128, KC * R], F32R, tag="xq", name="xq")
            for k in range(KC):
                nc.sync.dma_start(
                    xq[:, k * R:(k + 1) * R],
                    xT[k * 128:(k + 1) * 128, c0:c0 + R],
                )

            # hT = W1flat.T @ xT, then gelu(. + b1) -> ha
            ha = hapool.tile([128, MC * R], F32R, tag="ha", name="ha")
            for m in range(MC):
                hp = ps_h.tile([128, R], F32, tag="hp", name="hp")
                for k in range(KC):
                    nc.tensor.matmul(
                        hp[:],
                        lhsT=w1_sb[:, k * EH + m * 128: k * EH + (m + 1) * 128],
                        rhs=xq[:, k * R:(k + 1) * R],
                        start=(k == 0),
                        stop=(k == KC - 1),
                    )
                nc.scalar.activation(
                    ha[:, m * R:(m + 1) * R], hp[:], act_fn,
                    bias=b1_sb[:, m:m + 1], scale=1.0,
                )

            # gating logits gT = Wg @ xT
            gp = ps_g.tile([N_EXPERTS, R], F32, tag="gp", name="gp")
            for k in range(KC):
                nc.tensor.matmul(
                    gp[:],
                    lhsT=wg_sb[:, k * 32:(k + 1) * 32],
                    rhs=xq[:, k * R:(k + 1) * R],
                    start=(k == 0),
                    stop=(k == KC - 1),
                )

            # expert outputs o2T = W2blockdiag.T @ haT
            op = ps_o.tile([N_EXPERTS, R], F32, tag="op", name="op")
            for m in range(MC):
                nc.tensor.matmul(
                    op[:],
                    lhsT=w2_sb[:, m * 32:(m + 1) * 32],
                    rhs=ha[:, m * R:(m + 1) * R],
                    start=(m == 0),
                    stop=(m == MC - 1),
                )

            # st[0:32] = exp(g); st[32:64] = (o2 + b2) * exp(g)
            st = stpool.tile([64, R], F32R, tag="st", name="st")
            nc.scalar.activation(st[0:32, :], gp[:], AF.Exp)
            nc.vector.scalar_tensor_tensor(
                st[32:64, :], op[:], b2_sb[:], st[0:32, :], ALU.add, ALU.mult,
            )

            # partition-reduce: sums[0]=sum(exp), sums[1]=sum((o2+b2)*exp)
            sp = ps_s.tile([33, R], F32, tag="sp", name="sp")
            nc.tensor.matmul(
                sp[:], lhsT=sel_sb[:], rhs=st[:],
                start=True, stop=True,
            )

            rc = xopool.tile([1, R], F32, tag="rc", name="rc")
            xo = xopool.tile([1, R], F32, tag="xo", name="xo")
            nc.vector.reciprocal(rc[:], sp[0:1, :])
            nc.vector.tensor_mul(xo[:], sp[32:33, :], rc[:])
            nc.sync.dma_start(outT[0:1, c0:c0 + R], xo[:])

    nc.compile()
    return nc


def prep_weights(Wg, W1, b1, W2, b2):
    w1t = np.ascontiguousarray(
        np.asarray(W1, dtype=np.float32).transpose(1, 0, 2).reshape(D_IN, EH))
    wgt = np.ascontiguousarray(np.asarray(Wg, dtype=np.float32).T)
    w2bd = np.zeros((EH, N_EXPERTS), np.float32)
    W2 = np.asarray(W2, dtype=np.float32)
    for e in range(N_EXPERTS):
        w2bd[e * HID:(e + 1) * HID, e] = W2[e]
    b1c = np.ascontiguousarray(
        np.asarray(b1, dtype=np.float32).reshape(EH).reshape(MC, 128).T)
    b2c = np.asarray(b2, dtype=np.float32).reshape(N_EXPERTS, 1)
    selm = np.zeros((64, 33), np.float32)
    selm[0:32, 0] = 1.0
    selm[32:64, 32] = 1.0
    return {"w1t": w1t, "wgt": wgt, "w2bd": w2bd, "b1c": b1c, "b2c": b2c,
            "sel": selm}


class Runner:
    """Reusable SPMD executor: the multi-core path of
    concourse.bass2jax.run_bass_via_pjrt, factored so the jitted callable and
    device-resident inputs can be reused across calls (for benchmarking)."""

    def __init__(self, nc, n_cores=N_CORES):
        b2j.install_neuronx_cc_hook()
        self.nc = nc
        self.n_cores = n_cores
        partition_name = (
            nc.partition_id_tensor.name if nc.partition_id_tensor else None
        )
        in_names, out_names, out_avals, zero_outs = [], [], [], []
        for alloc in nc.m.functions[0].allocations:
            if not isinstance(alloc, mybir.MemoryLocationSet):
                continue
            assert alloc.memorylocations
            name = alloc.memorylocations[0].name
            if alloc.kind == "ExternalInput":
                if name != partition_name:
                    in_names.append(name)
            elif alloc.kind == "ExternalOutput":
                out_names.append(name)
                shape = tuple(alloc.tensor_shape)
                dtype = mybir.dt.np(alloc.dtype)
                out_avals.append(jax.core.ShapedArray(shape, dtype))
                zero_outs.append(np.zeros(shape, dtype))
        self.in_names = list(in_names)
        self.out_names = out_names
        self.zero_outs = zero_outs
        n_params = len(in_names)
        n_outs = len(out_names)
        bind_names = in_names + out_names
        if partition_name is not None:
            bind_names.append(partition_name)

        def _body(*args):
            operands = list(args)
            if partition_name is not None:
                operands.append(b2j.partition_id_tensor())
            outs = b2j._bass_exec_p.bind(
                *operands,
                out_avals=tuple(out_avals),
                in_names=tuple(bind_names),
                out_names=tuple(out_names),
                lowering_input_output_aliases=(),
                sim_require_finite=True,
                sim_require_nnan=True,
                nc=nc,
            )
            return tuple(outs)

        devices = jax.devices()[:n_cores]
        assert len(devices) == n_cores
        self.mesh = Mesh(np.asarray(devices), ("core",))
        in_specs = (PartitionSpec("core"),) * (n_params + n_outs)
        out_specs = (PartitionSpec("core"),) * n_outs
        self.fn = jax.jit(
            shard_map(_body, mesh=self.mesh, in_specs=in_specs,
                      out_specs=out_specs, check_rep=False),
            donate_argnums=tuple(range(n_params, n_params + n_outs)),
            keep_unused=True,
        )
        self.sharding = NamedSharding(self.mesh, PartitionSpec("core"))

    def put_inputs(self, in_maps):
        assert len(in_maps) == self.n_cores
        concat = [
            np.concatenate([np.asarray(m[name]) for m in in_maps], axis=0)
            for name in self.in_names
        ]
        return [jax.device_put(a, self.sharding) for a in concat]

    def fresh_outs(self):
        return [
            jax.device_put(
                np.zeros((self.n_cores * z.shape[0], *z.shape[1:]), z.dtype),
                self.sharding,
            )
            for z in self.zero_outs
        ]

    def run(self, dev_inputs, dev_outs=None):
        if dev_outs is None:
            dev_outs = self.fresh_outs()
        return self.fn(*dev_inputs, *dev_outs)


_RUNNER_CACHE = {}


def get_runner(b_loc=B_LOC):
    if b_loc not in _RUNNER_CACHE:
        if b_loc not in _NC_CACHE:
            _NC_CACHE[b_loc] = build_nc(b_loc)
        _RUNNER_CACHE[b_loc] = Runner(_NC_CACHE[b_loc])
    return _RUNNER_CACHE[b_loc]


def make_in_maps(x, Wg, W1, b1, W2, b2):
    x = np.asarray(x, dtype=np.float32)
    consts = prep_weights(Wg, W1, b1, W2, b2)
    xs = x.reshape(N_CORES, B_LOC, D_IN)
    in_maps = []
    for i in range(N_CORES):
        m = dict(consts)
        m["xT"] = np.ascontiguousarray(xs[i].T)
        in_maps.append(m)
    return in_maps


def kernel(x, Wg, W1, b1, W2, b2):
    os.environ["BASS_NEVER_TRACE"] = "1"
    in_maps = make_in_maps(x, Wg, W1, b1, W2, b2)
    runner = get_runner(B_LOC)
    dev_in = runner.put_inputs(in_maps)
    outs = runner.run(dev_in)
    out_t = np.asarray(outs[0])  # [N_CORES * 1, B_LOC]
    return np.ascontiguousarray(out_t.reshape(BATCH, 1))


if __name__ == "__main__":
    rng = np.random.default_rng(0)
    inputs = {
        "x": rng.standard_normal((BATCH, D_IN), dtype=np.float32),
        "Wg": (rng.standard_normal((N_EXPERTS, D_IN)) * 0.02).astype(np.float32),
        "W1": (rng.standard_normal((N_EXPERTS, D_IN, HID)) * 0.02).astype(np.float32),
        "b1": (rng.standard_normal((N_EXPERTS, HID)) * 0.02).astype(np.float32),
        "W2": (rng.standard_normal((N_EXPERTS, HID)) * 0.02).astype(np.float32),
        "b2": (rng.standard_normal((N_EXPERTS,)) * 0.02).astype(np.float32),
    }
    out = kernel(**inputs)
    print(out.shape, out.dtype, out[:4, 0])


# revision 11
# speedup vs baseline: 168.3067x; 1.1190x over previous
"""Trainium2 Bass kernel for MoEPred: softmax-gated mixture of 32 tiny experts.

  xi[b] = sum_e softmax_e(x@Wg.T) * (W2[e] . gelu(x @ W1[e] + b1[e]) + b2[e])

Sharding: pure data parallel over batch across 8 NeuronCores; weights are
replicated. x is pre-laid-out on the host so each 512-row macro-tile is one
contiguous 1MB DMA landing as xT chunks [feat 128, rows 512] (the contraction
dim on SBUF partitions). All matmuls run in float32r (full-rate fp32 PE mode).

Per 512-row macro-tile s (software-pipelined across 3 steps so the PE never
waits on ACT/DVE results of the same step):
  step s:   hT [512eh, R] = W1flat.T @ xT   (16 f32r matmuls)   [PE]
            haT = gelu(hT + b1)             (fused bias)        [ACT]
            gT [32, R] = Wg @ xT            (4 matmuls)         [PE]
            st[0:32] = exp(gT)                                  [ACT]
  step s+1: o2T [32, R] = W2bd.T @ haT      (4 matmuls)         [PE]
            st[32:64] = (o2T + b2) * st[0:32]                   [DVE]
  step s+2: sums [33, R] = sel.T @ st       (partition reduce)  [PE]
            xiT [1, R] = sums[32] * recip(sums[0])              [DVE]
"""

import os
import sys
from contextlib import ExitStack

import numpy as np

for _p in ("/opt/trn_rl_repo",):
    if _p not in sys.path:
        sys.path.insert(0, _p)

import jax
from jax.experimental.shard_map import shard_map
from jax.sharding import Mesh, NamedSharding, PartitionSpec

import concourse.bacc as bacc
import concourse.bass2jax as b2j
import concourse.tile as tile
from concourse import mybir

N_CORES = 8
BATCH = 262144
D_IN = 512
N_EXPERTS = 32
HID = 16
EH = N_EXPERTS * HID  # 512
B_LOC = BATCH // N_CORES  # 32768
R = 512  # rows per macro-tile
KC = D_IN // 128  # 4 feature chunks
MC = EH // 128  # 4 eh chunks

F32 = mybir.dt.float32
F32R = mybir.dt.float32r
AF = mybir.ActivationFunctionType
ALU = mybir.AluOpType

_NC_CACHE = {}
_RUNNER_CACHE = {}


def build_nc(b_loc=B_LOC, act_fn=None, loop_n=1, level=3):
    """loop_n > 1 wraps the macro loop in a hardware For_i that redoes the
    identical work loop_n times (benchmark amplification above the ~80-100ms
    axon dispatch floor). level: 0=dma only, 1=+mm1, 2=+gelu, 3=full."""
    if act_fn is None:
        act_fn = AF.Gelu
    assert b_loc % R == 0
    n_macro = b_loc // R

    nc = bacc.Bacc("TRN2", target_bir_lowering=False, debug=False, num_devices=N_CORES)

    xTm = nc.dram_tensor("xTm", [n_macro * 128, KC * R], F32R, kind="ExternalInput")
    w1t = nc.dram_tensor("w1t", [D_IN, EH], F32R, kind="ExternalInput")
    wgt = nc.dram_tensor("wgt", [D_IN, N_EXPERTS], F32R, kind="ExternalInput")
    w2bd = nc.dram_tensor("w2bd", [EH, N_EXPERTS], F32R, kind="ExternalInput")
    b1c = nc.dram_tensor("b1c", [128, MC], F32, kind="ExternalInput")
    b2c = nc.dram_tensor("b2c", [N_EXPERTS, 1], F32, kind="ExternalInput")
    sel = nc.dram_tensor("sel", [64, 33], F32R, kind="ExternalInput")
    outT = nc.dram_tensor("outT", [1, b_loc], F32, kind="ExternalOutput")

    with tile.TileContext(nc) as tc, ExitStack() as ctx:
        const = ctx.enter_context(tc.tile_pool(name="const", bufs=1))
        xpool = ctx.enter_context(tc.tile_pool(name="xp", bufs=3))
        hapool = ctx.enter_context(tc.tile_pool(name="hap", bufs=2))
        stpool = ctx.enter_context(tc.tile_pool(name="stp", bufs=3))
        xopool = ctx.enter_context(tc.tile_pool(name="xop", bufs=4))
        ps_h = ctx.enter_context(tc.tile_pool(name="ps_h", bufs=2, space="PSUM"))
        ps_g = ctx.enter_context(tc.tile_pool(name="ps_g", bufs=2, space="PSUM"))
        ps_o = ctx.enter_context(tc.tile_pool(name="ps_o", bufs=2, space="PSUM"))
        ps_s = ctx.enter_context(tc.tile_pool(name="ps_s", bufs=2, space="PSUM"))

        # --- replicated constants, loaded once ---
        w1_sb = const.tile([128, KC * EH], F32R, name="w1_sb")
        wg_sb = const.tile([128, KC * N_EXPERTS], F32R, name="wg_sb")
        w2_sb = const.tile([128, MC * N_EXPERTS], F32R, name="w2_sb")
        b1_sb = const.tile([128, MC], F32, name="b1_sb")
        b2_sb = const.tile([N_EXPERTS, 1], F32, name="b2_sb")
        sel_sb = const.tile([64, 33], F32R, name="sel_sb")
        for k in range(KC):
            nc.sync.dma_start(w1_sb[:, k * EH:(k + 1) * EH], w1t[k * 128:(k + 1) * 128, :])
            nc.sync.dma_start(wg_sb[:, k * 32:(k + 1) * 32], wgt[k * 128:(k + 1) * 128, :])
            nc.sync.dma_start(w2_sb[:, k * 32:(k + 1) * 32], w2bd[k * 128:(k + 1) * 128, :])
        nc.sync.dma_start(b1_sb[:], b1c[:, :])
        nc.sync.dma_start(b2_sb[:], b2c[:, :])
        nc.sync.dma_start(sel_sb[:], sel[:, :])

        if loop_n > 1:
            ctx.enter_context(tc.For_i(0, loop_n, 1))

        # pipeline state: stage1[j] = (ha, st), stage2[j] = st
        pend1 = []  # (j, ha, st): needs MM2 + stt
        pend2 = []  # (j, st): needs sums + div + out dma

        def emit_stage1_tail():
            jj, ha_p, st_p = pend1.pop(0)
            op = ps_o.tile([N_EXPERTS, R], F32, tag="op", name="op")
            for m in range(MC):
                nc.tensor.matmul(op[:], lhsT=w2_sb[:, m * 32:(m + 1) * 32],
                                 rhs=ha_p[:, m * R:(m + 1) * R],
                                 start=(m == 0), stop=(m == MC - 1))
            nc.vector.scalar_tensor_tensor(
                st_p[32:64, :], op[:], b2_sb[:], st_p[0:32, :], ALU.add, ALU.mult)
            pend2.append((jj, st_p))

        def emit_stage2_tail():
            jj, st_p = pend2.pop(0)
            sp = ps_s.tile([33, R], F32, tag="sp", name="sp")
            nc.tensor.matmul(sp[:], lhsT=sel_sb[:], rhs=st_p[:], start=True, stop=True)
            rc = xopool.tile([1, R], F32, tag="rc", name="rc")
            xo = xopool.tile([1, R], F32, tag="xo", name="xo")
            nc.vector.reciprocal(rc[:], sp[0:1, :])
            nc.vector.tensor_mul(xo[:], sp[32:33, :], rc[:])
            nc.sync.dma_start(outT[0:1, jj * R:(jj + 1) * R], xo[:])

        for j in range(n_macro):
            xq = xpool.tile([128, KC * R], F32R, tag="xq", name="xq")
            nc.sync.dma_start(xq[:], xTm[j * 128:(j + 1) * 128, :])

            if level >= 1:
                # hT = W1flat.T @ xT ; gelu(hT + b1) -> ha
                ha = hapool.tile([128, MC * R], F32R, tag="ha", name="ha")
                for m in range(MC):
                    hp = ps_h.tile([128, R], F32, tag="hp", name="hp")
                    for k in range(KC):
                        nc.tensor.matmul(
                            hp[:],
                            lhsT=w1_sb[:, k * EH + m * 128: k * EH + (m + 1) * 128],
                            rhs=xq[:, k * R:(k + 1) * R],
                            start=(k == 0), stop=(k == KC - 1))
                    if level >= 2:
                        nc.scalar.activation(
                            ha[:, m * R:(m + 1) * R], hp[:], act_fn,
                            bias=b1_sb[:, m:m + 1], scale=1.0)

            if level >= 3:
                # gating logits + exp
                gp = ps_g.tile([N_EXPERTS, R], F32, tag="gp", name="gp")
                for k in range(KC):
                    nc.tensor.matmul(gp[:], lhsT=wg_sb[:, k * 32:(k + 1) * 32],
                                     rhs=xq[:, k * R:(k + 1) * R],
                                     start=(k == 0), stop=(k == KC - 1))
                st = stpool.tile([64, R], F32R, tag="st", name="st")
                nc.scalar.activation(st[0:32, :], gp[:], AF.Exp)
                pend1.append((j, ha, st))

                if j >= 1:
                    emit_stage1_tail()
                if j >= 2:
                    emit_stage2_tail()
            else:
                xo = xopool.tile([1, R], F32, tag="xo", name="xo")
                nc.vector.memset(xo[:], 0.0)
                nc.sync.dma_start(outT[0:1, j * R:(j + 1) * R], xo[:])

        # drain the pipeline
        while pend1:
            emit_stage1_tail()
        while pend2:
            emit_stage2_tail()

    nc.compile()
    return nc


def prep_weights(Wg, W1, b1, W2, b2):
    w1t = np.ascontiguousarray(
        np.asarray(W1, dtype=np.float32).transpose(1, 0, 2).reshape(D_IN, EH))
    wgt = np.ascontiguousarray(np.asarray(Wg, dtype=np.float32).T)
    w2bd = np.zeros((EH, N_EXPERTS), np.float32)
    W2 = np.asarray(W2, dtype=np.float32)
    for e in range(N_EXPERTS):
        w2bd[e * HID:(e + 1) * HID, e] = W2[e]
    b1c = np.ascontiguousarray(
        np.asarray(b1, dtype=np.float32).reshape(EH).reshape(MC, 128).T)
    b2c = np.asarray(b2, dtype=np.float32).reshape(N_EXPERTS, 1)
    selm = np.zeros((64, 33), np.float32)
    selm[0:32, 0] = 1.0
    selm[32:64, 32] = 1.0
    return {"w1t": w1t, "wgt": wgt, "w2bd": w2bd, "b1c": b1c, "b2c": b2c,
            "sel": selm}


def layout_x(xc):
    """Core shard [B_LOC, D_IN] -> per-macro contiguous transposed layout
    [n_macro*128, KC*R]: xTm[j*128+p, k*R+c] = xc[j*R+c, k*128+p]."""
    n_macro = xc.shape[0] // R
    return np.ascontiguousarray(
        xc.reshape(n_macro, R, KC, 128).transpose(0, 3, 2, 1).reshape(
            n_macro * 128, KC * R))


class Runner:
    """Reusable SPMD executor: the multi-core path of
    concourse.bass2jax.run_bass_via_pjrt, factored so the jitted callable and
    device-resident inputs can be reused across calls (for benchmarking)."""

    def __init__(self, nc, n_cores=N_CORES):
        b2j.install_neuronx_cc_hook()
        self.nc = nc
        self.n_cores = n_cores
        partition_name = (
            nc.partition_id_tensor.name if nc.partition_id_tensor else None
        )
        in_names, out_names, out_avals, zero_outs = [], [], [], []
        for alloc in nc.m.functions[0].allocations:
            if not isinstance(alloc, mybir.MemoryLocationSet):
                continue
            assert alloc.memorylocations
            name = alloc.memorylocations[0].name
            if alloc.kind == "ExternalInput":
                if name != partition_name:
                    in_names.append(name)
            elif alloc.kind == "ExternalOutput":
                out_names.append(name)
                shape = tuple(alloc.tensor_shape)
                dtype = mybir.dt.np(alloc.dtype)
                out_avals.append(jax.core.ShapedArray(shape, dtype))
                zero_outs.append(np.zeros(shape, dtype))
        self.in_names = list(in_names)
        self.out_names = out_names
        self.zero_outs = zero_outs
        n_params = len(in_names)
        n_outs = len(out_names)
        bind_names = in_names + out_names
        if partition_name is not None:
            bind_names.append(partition_name)

        def _body(*args):
            operands = list(args)
            if partition_name is not None:
                operands.append(b2j.partition_id_tensor())
            outs = b2j._bass_exec_p.bind(
                *operands,
                out_avals=tuple(out_avals),
                in_names=tuple(bind_names),
                out_names=tuple(out_names),
                lowering_input_output_aliases=(),
                sim_require_finite=True,
                sim_require_nnan=True,
                nc=nc,
            )
            return tuple(outs)

        devices = jax.devices()[:n_cores]
        assert len(devices) == n_cores
        self.mesh = Mesh(np.asarray(devices), ("core",))
        in_specs = (PartitionSpec("core"),) * (n_params + n_outs)
        out_specs = (PartitionSpec("core"),) * n_outs
        self.fn = jax.jit(
            shard_map(_body, mesh=self.mesh, in_specs=in_specs,
                      out_specs=out_specs, check_rep=False),
            donate_argnums=tuple(range(n_params, n_params + n_outs)),
            keep_unused=True,
        )
        self.sharding = NamedSharding(self.mesh, PartitionSpec("core"))

    def put_inputs(self, in_maps):
        assert len(in_maps) == self.n_cores
        concat = [
            np.concatenate([np.asarray(m[name]) for m in in_maps], axis=0)
            for name in self.in_names
        ]
        return [jax.device_put(a, self.sharding) for a in concat]

    def fresh_outs(self):
        return [
            jax.device_put(
                np.zeros((self.n_cores * z.shape[0], *z.shape[1:]), z.dtype),
                self.sharding,
            )
            for z in self.zero_outs
        ]

    def run(self, dev_inputs, dev_outs=None):
        if dev_outs is None:
            dev_outs = self.fresh_outs()
        return self.fn(*dev_inputs, *dev_outs)


def get_runner(b_loc=B_LOC):
    if b_loc not in _RUNNER_CACHE:
        if b_loc not in _NC_CACHE:
            _NC_CACHE[b_loc] = build_nc(b_loc)
        _RUNNER_CACHE[b_loc] = Runner(_NC_CACHE[b_loc])
    return _RUNNER_CACHE[b_loc]


def make_in_maps(x, Wg, W1, b1, W2, b2):
    x = np.asarray(x, dtype=np.float32)
    consts = prep_weights(Wg, W1, b1, W2, b2)
    xs = x.reshape(N_CORES, B_LOC, D_IN)
    in_maps = []
    for i in range(N_CORES):
        m = dict(consts)
        m["xTm"] = layout_x(xs[i])
        in_maps.append(m)
    return in_maps


def kernel(x, Wg, W1, b1, W2, b2):
    os.environ["BASS_NEVER_TRACE"] = "1"
    in_maps = make_in_maps(x, Wg, W1, b1, W2, b2)
    runner = get_runner(B_LOC)
    dev_in = runner.put_inputs(in_maps)
    outs = runner.run(dev_in)
    out_t = np.asarray(outs[0])  # [N_CORES * 1, B_LOC]
    return np.ascontiguousarray(out_t.reshape(BATCH, 1))


if __name__ == "__main__":
    rng = np.random.default_rng(0)
    inputs = {
        "x": rng.standard_normal((BATCH, D_IN), dtype=np.float32),
        "Wg": (rng.standard_normal((N_EXPERTS, D_IN)) * 0.02).astype(np.float32),
        "W1": (rng.standard_normal((N_EXPERTS, D_IN, HID)) * 0.02).astype(np.float32),
        "b1": (rng.standard_normal((N_EXPERTS, HID)) * 0.02).astype(np.float32),
        "W2": (rng.standard_normal((N_EXPERTS, HID)) * 0.02).astype(np.float32),
        "b2": (rng.standard_normal((N_EXPERTS,)) * 0.02).astype(np.float32),
    }
    out = kernel(**inputs)
    print(out.shape, out.dtype, out[:4, 0])
